# revision 41
# baseline (speedup 1.0000x reference)
"""Trainium2 Bass kernel for nn_CausalSelfAttention_16810501996824.

Head-sharded (tensor-parallel) causal self-attention over 8 NeuronCores:
each core owns 2 of the 16 heads through QKV projection, RMS norm, rotary,
causal attention with sigmoid gate and lambda-blended V. The per-head
context vectors are then exchanged with a single AllToAll (t-sharded), and
each core computes a 256-row t-slice of the full output projection.

bf16 datapath with fp32 PSUM accumulation. The sigmoid gate is computed on
the host (input-only), the lambda blend is folded into Wv/v1, and rsqrt is
exp(-0.5*ln(x)) so the scalar engine never switches activation table sets.

Self-contained: hardcodes all shapes; builds + compiles the Bass module on
first call and caches the jitted SPMD executable.
"""
import json

import numpy as np

# ---------------------------------------------------------------------------
# Problem constants
# ---------------------------------------------------------------------------
DIM = 1024
N_HEAD = 16
T = 2048
HD = 64                 # head dim
GATE_IN = 12
ROPE_BASE = 10000.0
ATTN_SCALE = 0.1
EPS = 1e-6
N_CORES = 8
HPC = N_HEAD // N_CORES  # heads per core = 2
C = HPC * HD             # channels per core = 128
NT512 = T // 512         # 4 t-windows
NS128 = T // 128         # 16 s-blocks
TS = T // N_CORES        # 256: per-core output t-slice

# ---------------------------------------------------------------------------
# Workaround: the staged walrus build allows at most 1 sem wait per
# instruction (2 for EventSemaphore); stock Tile piles multiple waits onto
# one instruction. Split extras onto single-wait NoOps at serialization.
# ---------------------------------------------------------------------------
_WAIT_CAP = {"EventSemaphore": 2}


def _split_multi_waits(bir: dict) -> dict:
    for fn in bir.get("functions", []):
        for blk in fn.get("blocks", []):
            out = []
            changed = False
            for inst in blk.get("instructions", []):
                si = inst.get("sync_info") or {}
                waits = si.get("on_wait") or []
                cap = _WAIT_CAP.get(inst.get("opcode"), 1)
                if len(waits) > cap:
                    changed = True
                    for j, w in enumerate(waits[cap:]):
                        out.append({
                            "debug": inst.get("debug", 0),
                            "engine": inst["engine"],
                            "ins": [], "outs": [],
                            "name": f"{inst['name']}-wsplit{j}",
                            "opcode": "NoOp",
                            "sync_info": {"on_update": [], "on_wait": [w]},
                            "text_hint": "wait_split",
                        })
                    si = dict(si)
                    si["on_wait"] = waits[:cap]
                    inst = dict(inst)
                    inst["sync_info"] = si
                out.append(inst)
            if changed:
                blk["instructions"] = out
    return bir


def _install_patches():
    import concourse.bass as bass
    if getattr(bass.Bass, "_wait_split_patched", False):
        return
    orig = bass.Bass.to_json_bytes

    def patched(self, *a, **k):
        return json.dumps(_split_multi_waits(json.loads(orig(self, *a, **k)))).encode()

    bass.Bass.to_json_bytes = patched
    bass.Bass._wait_split_patched = True


# ---------------------------------------------------------------------------
# Bass module
# ---------------------------------------------------------------------------

def _build_module(repeat=1, phases=4):
    import concourse.bass as bass
    import concourse.mybir as mybir
    import concourse.tile as tile

    F32 = mybir.dt.float32
    BF16 = mybir.dt.bfloat16
    AF = mybir.ActivationFunctionType

    nc = bass.Bass()

    # host pre-blocked: xblk[p, (tj, d, t512)] and wblk[p, (d, m384)]
    xblk = nc.declare_dram_parameter("xblk", [128, NT512 * 8 * 512], BF16,
                                     isOutput=False)
    wblk = nc.declare_dram_parameter("wblk", [128, 8 * 3 * C], BF16,
                                     isOutput=False)
    gated = nc.declare_dram_parameter("gated", [HPC, T], BF16, isOutput=False)
    wprojT = nc.declare_dram_parameter("wprojT", [DIM, DIM], BF16, isOutput=False)
    v1blk = nc.declare_dram_parameter("v1blk", [128, NS128 * C], BF16, isOutput=False)
    cosd = nc.declare_dram_parameter("cosd", [C, T], BF16, isOutput=False)
    sind = nc.declare_dram_parameter("sind", [C, T], BF16, isOutput=False)
    swapm = nc.declare_dram_parameter("swapm", [128, 128], BF16, isOutput=False)
    identm = nc.declare_dram_parameter("identm", [128, 128], BF16, isOutput=False)
    bsumd = nc.declare_dram_parameter("bsumd", [128, 2], BF16, isOutput=False)
    mqkd = nc.declare_dram_parameter("mqkd", [2, 128], BF16, isOutput=False)
    outT = nc.declare_dram_parameter("outT", [DIM, TS], F32, isOutput=True)

    # t-shard exchange: core r owns t-slice [256r, 256r+256); shard j of
    # y_loc goes to core j (single AllToAll after phase 2).
    y_loc = nc.dram_tensor("y_loc", [N_CORES, C, TS], BF16)
    y_recv = nc.dram_tensor("y_recv", [N_CORES, C, TS], BF16)

    with nc.allow_low_precision(reason="bf16 matmul pipeline"), \
            tile.TileContext(nc) as tc:
      with tc.tile_pool(name="const", bufs=1) as persist, \
           tc.tile_pool(name="vaug", bufs=1) as vaug_pool:
        # ---- tiles persistent across repeats: constants load once; qt/kt/
        # vt/v_aug are fully rewritten each repeat, so sharing them lets the
        # scheduler pipeline consecutive repeats instead of reallocating ----
        qt = persist.tile([128, T], BF16)      # normalized+rotated qT
        kt = persist.tile([128, T], BF16)
        vt = persist.tile([128, T], BF16)      # vT ((1-lam) pre-scaled)
        gate_sb = [persist.tile([1, T], BF16, name=f"gate{h}", tag=f"gate{h}")
                   for h in range(HPC)]
        cos_sb = persist.tile([C, T], BF16)
        sin_sb = persist.tile([C, T], BF16)
        ident = persist.tile([128, 128], BF16)
        swp = persist.tile([128, 128], BF16)
        bsum = persist.tile([128, 2], BF16)
        mqk = persist.tile([2, 128], BF16)
        ones_row = persist.tile([1, 64], BF16)
        eps_sb = persist.tile([128, 1], F32)
        nc.vector.memset(eps_sb, EPS)
        wp = [persist.tile([128, DIM], BF16, name=f"wp{i}", tag=f"wp{i}")
              for i in range(8)]
        w_sb = persist.tile([128, 8 * 3 * C], BF16)
        v_aug = [[vaug_pool.tile([128, HD + 1], BF16, name=f"va{h}_{si}",
                                 tag=f"va{h}_{si}")
                  for si in range(NS128)] for h in range(HPC)]

        v1l_sb = persist.tile([128, NS128 * C], BF16)
        # small constants + big side-loads on the ACT HWDGE queue; x and
        # wqkv (phase-1 critical path) go on the SP queue; wproj prefetch
        # on the gpsimd SWDGE queue (all consumed only in phase 4).
        nc.scalar.dma_start(out=swp, in_=swapm[:])
        nc.scalar.dma_start(out=ident, in_=identm[:])
        nc.scalar.dma_start(out=bsum, in_=bsumd[:])
        nc.scalar.dma_start(out=mqk, in_=mqkd[:])
        for h in range(HPC):
            nc.scalar.dma_start(out=gate_sb[h], in_=gated[h:h + 1, :])
        nc.scalar.dma_start(out=cos_sb, in_=cosd[:])
        nc.scalar.dma_start(out=sin_sb, in_=sind[:])
        nc.scalar.dma_start(out=v1l_sb, in_=v1blk[:])
        nc.sync.dma_start(out=w_sb, in_=wblk[:])
        nc.vector.memset(ones_row, 1.0)
        for h in range(HPC):
            for si in range(NS128):
                nc.vector.memset(v_aug[h][si][:, HD:HD + 1], 1.0)
        for i in range(8):
            nc.gpsimd.dma_start(out=wp[i], in_=wprojT[128 * i:128 * (i + 1), :])
        # 0/1 causal masks for diagonal offsets 1 and 3 (DVE path; offsets
        # 0 and 2 are masked in-place on gpsimd)
        dmask = {}
        for ko in (1, 3):
            dm = persist.tile([128, 512], BF16, name=f"dm{ko}", tag=f"dm{ko}")
            dmask[ko] = dm
            nc.vector.memset(dm, 1.0)
            nc.gpsimd.affine_select(
                out=dm, in_=dm,
                compare_op=mybir.AluOpType.is_ge,
                fill=0.0, base=-(128 * ko),
                channel_multiplier=-1, pattern=[[1, 512]])

        for _rep in range(repeat):
          if True:

            # =============================================================
            # Phase 1: QKV projections + RMS norm + rotary
            # =============================================================
            with tc.tile_pool(name="p1x", bufs=3) as p1x, \
                 tc.tile_pool(name="p1t", bufs=3) as p1t, \
                 tc.tile_pool(name="p1acc", bufs=1, space="PSUM") as p1acc, \
                 tc.tile_pool(name="p1aux", bufs=3, space="PSUM") as p1aux, \
                 tc.tile_pool(name="p1small", bufs=2, space="PSUM") as p1s:
                wts = [w_sb[:, 384 * d:384 * (d + 1)] for d in range(8)]

                for tj in range(NT512):
                    ts = slice(512 * tj, 512 * (tj + 1))
                    x_sb = p1x.tile([128, 8 * 512], BF16, tag="x")
                    eng = nc.sync if tj % 2 == 0 else nc.scalar
                    eng.dma_start(out=x_sb,
                                  in_=xblk[:, 4096 * tj:4096 * (tj + 1)])
                    xts = [x_sb[:, 512 * d:512 * (d + 1)] for d in range(8)]

                    q_ps = p1acc.tile([128, 512], F32, tag="q_ps")
                    k_ps = p1acc.tile([128, 512], F32, tag="k_ps")
                    v_ps = p1acc.tile([128, 512], F32, tag="v_ps")
                    for d in range(8):
                        nc.tensor.matmul(q_ps, wts[d][:, 0:128], xts[d],
                                         start=(d == 0), stop=(d == 7))
                    for d in range(8):
                        nc.tensor.matmul(k_ps, wts[d][:, 128:256], xts[d],
                                         start=(d == 0), stop=(d == 7))
                    for d in range(8):
                        nc.tensor.matmul(v_ps, wts[d][:, 256:384], xts[d],
                                         start=(d == 0), stop=(d == 7))
                    nc.scalar.copy(vt[:, ts], v_ps)

                    for name, r_ps, dst in (("q", q_ps, qt), ("k", k_ps, kt)):
                        raw = p1t.tile([128, 512], BF16, tag=f"{name}raw")
                        nc.scalar.copy(raw, r_ps)
                        sq = p1t.tile([128, 512], BF16, tag=f"{name}sq")
                        nc.scalar.activation(sq, r_ps, AF.Square)
                        ms = p1s.tile([2, 512], F32, tag="ms")
                        nc.tensor.matmul(ms, bsum, sq, start=True, stop=True)
                        lms = p1t.tile([2, 512], F32, tag=f"{name}lms")
                        nc.scalar.activation(lms, ms, AF.Ln, bias=eps_sb[0:2, :])
                        ars = p1t.tile([2, 512], BF16, tag=f"{name}ars")
                        nc.scalar.activation(ars, lms, AF.Exp, scale=-0.5)
                        bc_ps = p1aux.tile([128, 512], F32, tag="aux")
                        nc.tensor.matmul(bc_ps, mqk, ars, start=True, stop=True)
                        sw_ps = p1aux.tile([128, 512], F32, tag="aux")
                        nc.tensor.matmul(sw_ps, swp, raw, start=True, stop=True)
                        t1 = p1t.tile([128, 512], BF16, tag=f"{name}t1")
                        nc.gpsimd.tensor_mul(t1, raw, cos_sb[:, ts])
                        t2 = p1t.tile([128, 512], BF16, tag=f"{name}t2")
                        nc.vector.tensor_mul(t2, sw_ps, sin_sb[:, ts])
                        t3 = p1t.tile([128, 512], BF16, tag=f"{name}t3")
                        nc.gpsimd.tensor_add(t3, t1, t2)
                        nc.vector.tensor_mul(dst[:, ts], t3, bc_ps)

                    # vT -> v_aug blocks for this window (v pre-scaled by
                    # (1-lam) on host; v1blk holds lam*v1 pre-blocked)
                    for si in range(4 * tj, 4 * (tj + 1)):
                        ss = slice(128 * si, 128 * (si + 1))
                        tr_ps = p1aux.tile([128, 128], BF16, tag="aux")
                        nc.tensor.transpose(tr_ps, vt[:, ss], ident)
                        for h in range(HPC):
                            nc.vector.tensor_add(
                                v_aug[h][si][:, 0:HD],
                                tr_ps[:, HD * h:HD * (h + 1)],
                                v1l_sb[:, C * si + HD * h:C * si + HD * (h + 1)])

            # =============================================================
            # Phase 2: causal attention per head
            # =============================================================
            if phases < 2:
                nc.sync.dma_start(out=outT[0:128, :].bitcast(BF16), in_=qt[:, 0:TS])
                continue
            with tc.tile_pool(name="p2t", bufs=4) as p2t, \
                 tc.tile_pool(name="p2small", bufs=2) as p2s, \
                 tc.tile_pool(name="sps", bufs=2, space="PSUM") as sps_pool, \
                 tc.tile_pool(name="yps", bufs=2, space="PSUM") as yps_pool, \
                 tc.tile_pool(name="bcps", bufs=2, space="PSUM") as bcps_pool:
                for tj in range(NT512):
                    ts = slice(512 * tj, 512 * (tj + 1))
                    for h in range(HPC):
                        hs = slice(HD * h, HD * (h + 1))
                        nsb = 4 * tj + 4
                        y_ps = yps_pool.tile([65, 512], F32, tag="y")
                        pending = None
                        for sp in range(nsb // 2):
                            s_ps = sps_pool.tile([128, 1024], F32, tag="s")
                            for k2 in range(2):
                                si = 2 * sp + k2
                                ss = slice(128 * si, 128 * (si + 1))
                                nc.tensor.matmul(s_ps[:, 512 * k2:512 * (k2 + 1)],
                                                 kt[hs, ss], qt[hs, ts],
                                                 start=True, stop=True)
                            p_sb = p2t.tile([128, 1024], BF16, tag="p")
                            nc.scalar.activation(p_sb, s_ps, AF.Exp,
                                                 scale=ATTN_SCALE)
                            for k2 in range(2):
                                si = 2 * sp + k2
                                ko = si - 4 * tj
                                if ko in (0, 2):
                                    nc.gpsimd.affine_select(
                                        out=p_sb[:, 512 * k2:512 * (k2 + 1)],
                                        in_=p_sb[:, 512 * k2:512 * (k2 + 1)],
                                        compare_op=mybir.AluOpType.is_ge,
                                        fill=0.0, base=-(128 * ko),
                                        channel_multiplier=-1, pattern=[[1, 512]])
                                elif ko in (1, 3):
                                    nc.vector.tensor_mul(
                                        p_sb[:, 512 * k2:512 * (k2 + 1)],
                                        p_sb[:, 512 * k2:512 * (k2 + 1)],
                                        dmask[ko])
                            for k2 in range(2):
                                si = 2 * sp + k2
                                if pending is not None:
                                    psi, pp = pending
                                    nc.tensor.matmul(y_ps, v_aug[h][psi], pp,
                                                     start=(psi == 0), stop=False)
                                pending = (si, p_sb[:, 512 * k2:512 * (k2 + 1)])
                        psi, pp = pending
                        nc.tensor.matmul(y_ps, v_aug[h][psi], pp,
                                         start=(psi == 0), stop=True)
                        pending = None
                        # normalize: yft = y * gate/denom (broadcast over hd)
                        u = p2s.tile([1, 512], F32, tag="u")
                        nc.vector.reciprocal(u, y_ps[64:65, :])
                        cs_row = p2s.tile([1, 512], BF16, tag="cs")
                        nc.vector.tensor_mul(cs_row, u, gate_sb[h][:, ts])
                        bc_ps = bcps_pool.tile([64, 512], F32, tag="bc")
                        nc.tensor.matmul(bc_ps, ones_row, cs_row,
                                         start=True, stop=True)
                        cs_sb = p2s.tile([64, 512], BF16, tag="csb")
                        nc.vector.tensor_copy(cs_sb, bc_ps)
                        yft = p2t.tile([64, 512], BF16, tag="yft")
                        nc.vector.tensor_mul(yft, y_ps[0:64, :], cs_sb)
                        for j2 in range(2):
                            sh = 2 * tj + j2
                            nc.scalar.dma_start(
                                out=y_loc[sh, 64 * h:64 * (h + 1), :],
                                in_=yft[:, 256 * j2:256 * (j2 + 1)])

            # =============================================================
            # Phase 3: AllToAll (t-shard exchange)
            # =============================================================
            if phases < 3:
                nc.sync.dma_start(out=outT[0:C, 0:128].bitcast(BF16),
                                  in_=y_loc[0][:])
                continue
            nc.gpsimd.collective_compute(
                "AllToAll", mybir.AluOpType.bypass,
                ins=[y_loc[:]], outs=[y_recv[:]],
                replica_groups=[list(range(N_CORES))],
            )

            # =============================================================
            # Phase 4: output projection for this core's 256-row t-slice
            # =============================================================
            if phases < 4:
                nc.sync.dma_start(out=outT[0:C, 0:128].bitcast(BF16),
                                  in_=y_recv[0][:])
                continue
            with tc.tile_pool(name="p4", bufs=1) as p4, \
                 tc.tile_pool(name="p4o", bufs=1) as p4o, \
                 tc.tile_pool(name="ops", bufs=1, space="PSUM") as ops_pool:
                yr = [p4.tile([128, TS], BF16, name=f"yr{i}", tag=f"yr{i}")
                      for i in range(8)]
                for i in range(8):
                    nc.sync.dma_start(out=yr[i], in_=y_recv[i][:])
                o_ps = [ops_pool.tile([128, TS], F32, name=f"o{j}", tag=f"o{j}")
                        for j in range(8)]
                for i in range(8):
                    for j in range(8):
                        nc.tensor.matmul(o_ps[j], wp[i][:, 128 * j:128 * (j + 1)],
                                         yr[i], start=(i == 0), stop=(i == 7))
                o_sb = p4o.tile([128, 8 * TS], F32, tag="osb")
                for j in range(8):
                    nc.scalar.copy(o_sb[:, TS * j:TS * (j + 1)], o_ps[j])
                    nc.sync.dma_start(out=outT[128 * j:128 * (j + 1), :],
                                      in_=o_sb[:, TS * j:TS * (j + 1)])

    return nc


# ---------------------------------------------------------------------------
# Host-side prep + cached runner
# ---------------------------------------------------------------------------

def _bf16(a):
    import ml_dtypes
    return np.ascontiguousarray(np.asarray(a).astype(ml_dtypes.bfloat16))


def _rotary_tables():
    i = np.arange(0, HD, 2, dtype=np.float32)
    inv_freq = (np.float32(1.0) / np.power(np.float32(ROPE_BASE),
                                           i / np.float32(HD))).astype(np.float32)
    t = np.arange(T, dtype=np.float32)
    freqs = t[:, None] * inv_freq[None, :]          # [T, 32]
    cos = np.cos(freqs).astype(np.float32)
    sin = np.sin(freqs).astype(np.float32)
    half = HD // 2
    cosd = np.empty((C, T), np.float32)
    sind = np.empty((C, T), np.float32)
    for h in range(HPC):
        base = HD * h
        cosd[base:base + half] = cos.T
        cosd[base + half:base + HD] = cos.T
        sind[base:base + half] = sin.T
        sind[base + half:base + HD] = -sin.T
    return cosd, sind


def _swap_matrix():
    m = np.zeros((128, 128), np.float32)
    half = HD // 2
    for r in range(128):
        blk, off = divmod(r, HD)
        src = blk * HD + ((off + half) % HD)
        m[src, r] = 1.0
    return m


_CACHE = {}


def _get_runner(repeat=1, phases=4):
    key = f"runner{repeat}_{phases}"
    if key in _CACHE:
        return _CACHE[key]
    _install_patches()
    nc = _build_module(repeat, phases)

    import jax
    import concourse.mybir as mybir
    from jax.sharding import Mesh, PartitionSpec
    from jax.experimental.shard_map import shard_map
    from concourse import bass2jax

    bass2jax.install_neuronx_cc_hook()
    partition_name = nc.partition_id_tensor.name if nc.partition_id_tensor else None
    in_names, out_names, out_avals, zero_outs = [], [], [], []
    for alloc in nc.m.functions[0].allocations:
        if not isinstance(alloc, mybir.MemoryLocationSet):
            continue
        name = alloc.memorylocations[0].name
        if alloc.kind == "ExternalInput":
            if name != partition_name:
                in_names.append(name)
        elif alloc.kind == "ExternalOutput":
            shape = tuple(alloc.tensor_shape)
            dtype = mybir.dt.np(alloc.dtype)
            out_names.append(name)
            out_avals.append(jax.core.ShapedArray(shape, dtype))
            zero_outs.append(np.zeros(shape, dtype))
    all_in_names = in_names + out_names
    if partition_name is not None:
        all_in_names.append(partition_name)
    n_params, n_outs = len(in_names), len(out_avals)

    def _body(*args):
        operands = list(args)
        if partition_name is not None:
            operands.append(bass2jax.partition_id_tensor())
        return tuple(bass2jax._bass_exec_p.bind(
            *operands,
            out_avals=tuple(out_avals),
            in_names=tuple(all_in_names),
            out_names=tuple(out_names),
            lowering_input_output_aliases=(),
            sim_require_finite=True, sim_require_nnan=True, nc=nc,
        ))

    devices = jax.devices()[:N_CORES]
    mesh = Mesh(np.asarray(devices), ("core",))
    fn = jax.jit(
        shard_map(_body, mesh=mesh,
                  in_specs=(PartitionSpec("core"),) * (n_params + n_outs),
                  out_specs=(PartitionSpec("core"),) * n_outs,
                  check_rep=False),
        keep_unused=True,
    )
    state = {
        "fn": fn, "in_names": in_names, "out_names": out_names,
        "out_avals": out_avals, "zero_outs": zero_outs, "nc": nc,
    }
    _CACHE[key] = state
    return state


def _prep_inputs(x, v1, Wq, Wk, Wv, Wproj, lamb, Wgate):
    x = np.asarray(x, np.float32)
    v1 = np.asarray(v1, np.float32)
    lam = np.float32(np.asarray(lamb))
    xT = x[0].T                                      # [DIM, T]
    cosd, sind = _rotary_tables()
    swapm = _swap_matrix()
    # host-side gate: sigmoid(x[..., :12] @ Wgate.T) -> [T, N_HEAD]
    gate_all = 1.0 / (1.0 + np.exp(-(x[0, :, :GATE_IN] @ np.asarray(Wgate).T)))
    bsum = np.zeros((128, 2), np.float32)
    bsum[0:64, 0] = 1.0 / HD
    bsum[64:128, 1] = 1.0 / HD
    mqkm = np.zeros((2, 128), np.float32)
    mqkm[0, 0:64] = 1.0
    mqkm[1, 64:128] = 1.0
    in_maps = []
    # xblk[p, (tj, d, t512)] = xT[128*d + p, 512*tj + t512]
    xblk = _bf16(xT.reshape(8, 128, NT512, 512).transpose(1, 2, 0, 3)
                 .reshape(128, NT512 * 8 * 512))
    for r in range(N_CORES):
        rows = slice(C * r, C * (r + 1))
        heads = slice(HPC * r, HPC * (r + 1))
        wqkvT = np.concatenate(
            [np.asarray(Wq)[rows].T, np.asarray(Wk)[rows].T,
             (1.0 - lam) * np.asarray(Wv)[rows].T], axis=1).astype(np.float32)
        # wblk[p, (d, m)] = wqkvT[128*d + p, m]
        wblk = _bf16(wqkvT.reshape(8, 128, 3 * C).transpose(1, 0, 2)
                     .reshape(128, 8 * 3 * C))
        in_maps.append({
            "xblk": xblk,
            "wblk": wblk,
            "gated": _bf16(gate_all[:, heads].T),
            "wprojT": _bf16(np.asarray(Wproj).T),
            "v1blk": _bf16((lam * v1[0][:, rows]).reshape(NS128, 128, C)
                           .transpose(1, 0, 2).reshape(128, NS128 * C)),
            "cosd": _bf16(cosd),
            "sind": _bf16(sind),
            "swapm": _bf16(swapm),
            "identm": _bf16(np.eye(128, dtype=np.float32)),
            "bsumd": _bf16(bsum),
            "mqkd": _bf16(mqkm),
        })
    return in_maps


def _run(in_maps):
    st = _get_runner()
    concat_in = [
        np.ascontiguousarray(np.concatenate([in_maps[c][n] for c in range(N_CORES)],
                                            axis=0))
        for n in st["in_names"]
    ]
    concat_zeros = [
        np.zeros((N_CORES * z.shape[0], *z.shape[1:]), z.dtype)
        for z in st["zero_outs"]
    ]
    outs = st["fn"](*concat_in, *concat_zeros)
    outs = [np.asarray(o) for o in outs]
    return {n: outs[i].reshape(N_CORES, *st["out_avals"][i].shape)
            for i, n in enumerate(st["out_names"])}


def kernel(x, v1, Wq, Wk, Wv, Wproj, lamb, Wgate):
    in_maps = _prep_inputs(x, v1, Wq, Wk, Wv, Wproj, lamb, Wgate)
    res = _run(in_maps)
    outT = res["outT"]                               # [cores, DIM, TS]
    y = np.empty((1, T, DIM), np.float32)
    for r in range(N_CORES):
        y[0, TS * r:TS * (r + 1), :] = outT[r].T
    return y, np.asarray(v1, np.float32)
